# revision 59
# baseline (speedup 1.0000x reference)
"""Behavior-specific feed-forward (MoE routing) kernel for 8 Trainium2 cores.

Reference computes, for each token t with behavior b = type_seq[t]:
    out[t] = 0                                  if b == 0
    out[t] = LN(FFN_b(x[t]) + x[t])             if b in 1..NB
where FFN_b(x) = gelu(x @ W1[b] + b1[b]) @ W2[b] + b2[b], LN over d_model
with per-behavior gamma/beta.

Strategy: expert-parallel. Host routes tokens by type_seq: 2 cores per
behavior, each takes half that behavior's tokens (gathered + padded to a
multiple of 128). Each core runs a dense 512->2048->512 FFN + residual +
LayerNorm over its tokens with only its behavior's weights resident.
Host scatters results back; type-0 tokens stay zero.

Device kernel layout (per core):
  xt    [D, T]   activations transposed (d_model-major) - L1 matmul rhs
  resid [T, D]   gathered x (+ b2 folded in) token-major - residual add
  L1: psum[fchunk 128, tok 512] = sum_k W1[k,fchunk].T @ xt[k, tok]
      gelu+b1 on ScalarE -> hT sbuf [128, 16, tok]
  L2: psum[tok 128, D] = sum_kf hT[kf, tokchunk].T @ W2[kf, :]
      z = psum + resid; bn_stats/bn_aggr -> mean,var; normalize, *gamma+beta
"""

import math
import sys

import numpy as np

try:
    import concourse.bass as bass
except ImportError:
    sys.path.insert(0, "/opt/trn_rl_repo")
    import concourse.bass as bass

import concourse.mybir as mybir
import concourse.tile as tile
from concourse import bacc
from concourse.bass import ts
from concourse.bass_utils import run_bass_kernel_spmd

D_MODEL = 512
D_FF = 2048
N_BEHAVIORS = 4
N_CORES = 8
LN_EPS = 1e-12
P = 128
KD = D_MODEL // P  # 4 k-chunks for layer 1
KF = D_FF // P  # 16 k-chunks for layer 2
GRP = 512  # token group (matmul moving free dim)

# matmul dtype: "f32r" (full-rate fp32) or "bf16"
MM_DTYPE = "f32r"

_cache = {}


def _np_mm_dtype():
    if MM_DTYPE == "bf16":
        import ml_dtypes

        return np.dtype(ml_dtypes.bfloat16)
    return np.dtype(np.float32)


def _build(t_cap: int, ln_affine: bool = True):
    """Build the single-core Bass program for capacity t_cap tokens."""
    mmdt = mybir.dt.float32r if MM_DTYPE == "f32r" else mybir.dt.bfloat16
    f32 = mybir.dt.float32

    nc = bacc.Bacc("TRN2", target_bir_lowering=False)
    xt_d = nc.dram_tensor("xt", [D_MODEL, t_cap], mmdt, kind="ExternalInput")
    resid_d = nc.dram_tensor("resid", [t_cap, D_MODEL], f32, kind="ExternalInput")
    w1_d = nc.dram_tensor("w1", [D_MODEL, D_FF], mmdt, kind="ExternalInput")
    w2_d = nc.dram_tensor("w2", [D_FF, D_MODEL], mmdt, kind="ExternalInput")
    b1t_d = nc.dram_tensor("b1t", [P, KF], f32, kind="ExternalInput")
    gamma_d = nc.dram_tensor("gamma", [D_MODEL], f32, kind="ExternalInput")
    beta_d = nc.dram_tensor("beta", [D_MODEL], f32, kind="ExternalInput")
    out_d = nc.dram_tensor("out", [t_cap, D_MODEL], f32, kind="ExternalOutput")

    w1_r = w1_d[:].rearrange("(kd p) f -> p kd f", p=P)  # [P, KD, D_FF]
    w2_r = w2_d[:].rearrange("(kf p) d -> p kf d", p=P)  # [P, KF, D_MODEL]
    xt_r = xt_d[:].rearrange("(kd p) t -> p kd t", p=P)  # [P, KD, t_cap]

    n_grp = (t_cap + GRP - 1) // GRP

    with tile.TileContext(nc) as tc:
        with (
            tc.tile_pool(name="consts", bufs=1) as consts,
            tc.tile_pool(name="xt", bufs=3) as xt_pool,
            tc.tile_pool(name="ht", bufs=2) as ht_pool,
            tc.tile_pool(name="resid", bufs=3) as resid_pool,
            tc.tile_pool(name="zt", bufs=8) as z_pool,
            tc.tile_pool(name="ot", bufs=3) as o_pool,
            tc.tile_pool(name="small", bufs=8) as small,
            tc.tile_pool(name="ps", bufs=8, space="PSUM") as ps_pool,
        ):
            # one-time constants; weights split into per-chunk DMAs so the
            # first matmuls only gate on the chunk they read. Order matters:
            # the DMA engines are a serial ~360GB/s resource, so small
            # early-needed tensors (b1) must precede the weight bulk.
            b1_sb = consts.tile([P, KF], f32)
            nc.sync.dma_start(out=b1_sb, in_=b1t_d[:])
            # w1 split by (kd, mf-half) in the exact order the kd-outer
            # halves-of-8 L1 loop consumes it
            w1_sb = consts.tile([P, KD, D_FF], mmdt)
            for h in range(2):
                for kd in range(KD):
                    # first chunk split again so matmul #1 starts ~1.5us earlier
                    nq = 2 if (h == 0 and kd == 0) else 1
                    sz = 1024 // nq
                    for q in range(nq):
                        lo = h * 1024 + q * sz
                        nc.scalar.dma_start(
                            out=w1_sb[:, kd, lo : lo + sz],
                            in_=w1_r[:, kd, lo : lo + sz],
                        )
            w2_sb = consts.tile([P, KF, D_MODEL], mmdt)
            if ln_affine:
                gamma_sb = consts.tile([P, D_MODEL], f32)
                nc.scalar.dma_start(
                    out=gamma_sb,
                    in_=bass.AP(tensor=gamma_d, offset=0, ap=[[0, P], [1, D_MODEL]]),
                )
                beta_sb = consts.tile([P, D_MODEL], f32)
                nc.scalar.dma_start(
                    out=beta_sb,
                    in_=bass.AP(tensor=beta_d, offset=0, ap=[[0, P], [1, D_MODEL]]),
                )
            # magic constant for DVE Newton-rsqrt (keeps Sqrt off ScalarE so
            # its function table never leaves Gelu)
            rsqrt_c = consts.tile([P, 4], mybir.dt.uint32)
            nc.vector.memset(rsqrt_c, 0x5F3759DF)

            def emit_l1(g):
                """Layer 1 for group g: h = gelu(x @ W1 + b1), transposed."""
                g0 = g * GRP
                gsz = min(GRP, t_cap - g0)
                n_sub = (gsz + P - 1) // P
                xt_sb = xt_pool.tile([P, KD, GRP], mmdt, tag="xt")
                for kd in range(KD):
                    nc.sync.dma_start(
                        out=xt_sb[:, kd : kd + 1, :gsz],
                        in_=xt_r[:, kd : kd + 1, g0 : g0 + gsz],
                    )
                ht_sb = ht_pool.tile([P, KF, GRP], mmdt, tag="ht")
                # kd-outer over half-groups of mf: the first matmuls only
                # need w1 chunk kd=0, so PE starts as soon as it lands, and
                # 8 psums in flight give PE slack while later chunks stream
                for h in range(2):
                    pss = [
                        ps_pool.tile([P, GRP], f32, tag="ps", name=f"ps1_{h}_{j}")
                        for j in range(8)
                    ]
                    for kd in range(KD):
                        for j in range(8):
                            nc.tensor.matmul(
                                pss[j][:, :gsz],
                                lhsT=w1_sb[:, kd, ts(8 * h + j, P)],
                                rhs=xt_sb[:, kd, :gsz],
                                start=(kd == 0),
                                stop=(kd == KD - 1),
                            )
                    for j in range(8):
                        mf = 8 * h + j
                        nc.scalar.activation(
                            out=ht_sb[:, mf, :gsz],
                            in_=pss[j][:, :gsz],
                            func=mybir.ActivationFunctionType.Gelu,
                            bias=b1_sb[:, mf : mf + 1],
                            scale=1.0,
                        )
                return ht_sb, g0, gsz

            def emit_l2(ht_sb, g0, gsz):
                """Layer 2 + residual + layernorm per 128-token tile."""
                n_sub = (gsz + P - 1) // P
                # this group's residual (token-major) in one DMA; needed only
                # by the z-adds, so it rides behind W2 in the DMA stream
                r_sb = resid_pool.tile([P, 4, D_MODEL], f32, tag="resid")
                resid_r = resid_d[:].rearrange("(s p) d -> p s d", p=P)
                nc.sync.dma_start(
                    out=r_sb[:, :n_sub, :],
                    in_=resid_r[:, g0 // P : g0 // P + n_sub, :],
                )
                mul = mybir.AluOpType.mult
                # process subtiles in pairs: matmul+residual+stats for two
                # tiles, then one batched DVE Newton-rsqrt chain, then the
                # normalizes — keeps the tail chain short and overlapped
                for pb in range(0, n_sub, 1):
                    pn = min(1, n_sub - pb)
                    z_tiles = []
                    mv_g = small.tile([P, 2, 2], f32, tag="mv")
                    for mt in range(pb, pb + pn):
                        m0 = mt * P
                        ps2 = ps_pool.tile([P, D_MODEL], f32, tag="ps")
                        for kf in range(KF):
                            nc.tensor.matmul(
                                ps2[:, :],
                                lhsT=ht_sb[:, kf, m0 : m0 + P],
                                rhs=w2_sb[:, kf, :],
                                start=(kf == 0),
                                stop=(kf == KF - 1),
                            )

                        z_sb = z_pool.tile([P, D_MODEL], f32, tag="z")
                        nc.vector.tensor_add(z_sb, ps2[:, :], r_sb[:, mt, :])
                        z_tiles.append(z_sb)

                        stats = small.tile([P, 6], f32, tag="stats")
                        nc.vector.bn_stats(out=stats, in_=z_sb)
                        nc.vector.bn_aggr(out=mv_g[:, mt - pb, :], in_=stats)

                    # rstd for the pair, [128, pn]: Newton rsqrt on DVE
                    # (bit-trick seed + 2 iterations; ~4e-6 relative)
                    mean_g = mv_g[:, :pn, 0]
                    vpe = small.tile([P, 2], f32, tag="vpe")
                    nc.vector.tensor_scalar(
                        vpe[:, :pn], mv_g[:, :pn, 1], LN_EPS, None,
                        op0=mybir.AluOpType.add,
                    )
                    y = small.tile([P, 2], f32, tag="y")
                    nc.vector.tensor_scalar(
                        y[:, :pn].bitcast(mybir.dt.uint32),
                        vpe[:, :pn].bitcast(mybir.dt.uint32),
                        1, None,
                        op0=mybir.AluOpType.logical_shift_right,
                    )
                    nc.vector.tensor_tensor(
                        y[:, :pn].bitcast(mybir.dt.uint32),
                        rsqrt_c[:, :pn],
                        y[:, :pn].bitcast(mybir.dt.uint32),
                        op=mybir.AluOpType.subtract,
                    )
                    a = small.tile([P, 2], f32, tag="a")
                    for _ in range(2):
                        nc.vector.tensor_tensor(a[:, :pn], y[:, :pn], y[:, :pn], op=mul)
                        nc.vector.tensor_tensor(a[:, :pn], a[:, :pn], vpe[:, :pn], op=mul)
                        nc.vector.tensor_scalar(
                            a[:, :pn], a[:, :pn], -0.5, 1.5,
                            op0=mul, op1=mybir.AluOpType.add,
                        )
                        nc.vector.tensor_tensor(y[:, :pn], y[:, :pn], a[:, :pn], op=mul)
                    # mr = mean * rstd (subtracted per tile below)
                    nmr = small.tile([P, 2], f32, tag="nmr")
                    nc.vector.tensor_tensor(nmr[:, :pn], mean_g, y[:, :pn], op=mul)

                    for mt in range(pb, pb + pn):
                        m0 = mt * P
                        j = mt - pb
                        # normed = z*rstd - mean*rstd (one DVE tensor_scalar)
                        o_sb = o_pool.tile([P, D_MODEL], f32, tag="o")
                        nc.vector.tensor_scalar(
                            o_sb,
                            z_tiles[j],
                            y[:, j : j + 1],
                            nmr[:, j : j + 1],
                            op0=mul,
                            op1=mybir.AluOpType.subtract,
                        )
                        if ln_affine:
                            nc.vector.tensor_mul(o_sb, o_sb, gamma_sb)
                            nc.vector.tensor_add(o_sb, o_sb, beta_sb)

                        nc.sync.dma_start(
                            out=out_d[g0 + m0 : g0 + m0 + P, :], in_=o_sb
                        )

            # software-pipelined emission: L1 runs one group ahead of L2 so
            # the PE never stalls on W2's arrival or group transitions.
            # W2's bulk DMA is emitted after the first two groups' loads.
            pending = [emit_l1(g) for g in range(min(2, n_grp))]
            for kq in range(8):
                nc.scalar.dma_start(
                    out=w2_sb[:, 2 * kq : 2 * kq + 2, :],
                    in_=w2_r[:, 2 * kq : 2 * kq + 2, :],
                )
            for g in range(n_grp):
                emit_l2(*pending[g])
                if g + 2 < n_grp:
                    pending.append(emit_l1(g + 2))

    nc.compile()
    return nc


def _get_program(t_cap: int, ln_affine: bool = True):
    key = (t_cap, MM_DTYPE, ln_affine)
    if key not in _cache:
        _cache[key] = _build(t_cap, ln_affine)
    return _cache[key]


def _prepare(input_tensor, type_seq, W1, b1, W2, b2, gamma, beta):
    """Host-side routing: returns (in_maps, per_core_idx, shape, t_cap)."""
    x = np.ascontiguousarray(np.asarray(input_tensor, dtype=np.float32))
    tseq = np.asarray(type_seq).astype(np.int64)
    W1 = np.asarray(W1, dtype=np.float32)
    b1 = np.asarray(b1, dtype=np.float32)
    W2 = np.asarray(W2, dtype=np.float32)
    b2 = np.asarray(b2, dtype=np.float32)
    gamma = np.asarray(gamma, dtype=np.float32)
    beta = np.asarray(beta, dtype=np.float32)

    shape = x.shape
    xf = x.reshape(-1, D_MODEL)
    tf = tseq.reshape(-1)
    nb = W1.shape[0]
    cores_per_exp = N_CORES // nb

    per_core_idx = []
    for e in range(nb):
        idx = np.nonzero(tf == e + 1)[0]
        n = len(idx)
        for c in range(cores_per_exp):
            lo = (n * c) // cores_per_exp
            hi = (n * (c + 1)) // cores_per_exp
            per_core_idx.append((e, idx[lo:hi]))

    # round capacity to 256 so every group has moving dim >= 256 (f32r
    # matmuls drop to quarter rate below that)
    t_cap = max(256, int(math.ceil(max(len(i) for _, i in per_core_idx) / 256)) * 256)
    ln_affine = not (np.all(gamma == 1.0) and np.all(beta == 0.0))

    mmdt = _np_mm_dtype()
    in_maps = []
    for e, idx in per_core_idx:
        n = len(idx)
        xg = np.zeros((t_cap, D_MODEL), np.float32)
        xg[:n] = xf[idx]
        resid = xg.copy()
        resid[:n] += b2[e][None, :]
        in_maps.append(
            {
                "xt": np.ascontiguousarray(xg.T).astype(mmdt),
                "resid": resid,
                "w1": np.ascontiguousarray(W1[e]).astype(mmdt),
                "w2": np.ascontiguousarray(W2[e]).astype(mmdt),
                "b1t": np.ascontiguousarray(b1[e].reshape(KF, P).T),
                "gamma": gamma[e],
                "beta": beta[e],
            }
        )
    return in_maps, per_core_idx, shape, t_cap, ln_affine


def _scatter(results, per_core_idx, shape):
    out = np.zeros((shape[0] * shape[1], D_MODEL), np.float32)
    for core, (_, idx) in enumerate(per_core_idx):
        out[idx] = results[core]["out"][: len(idx)]
    return out.reshape(shape)


def run(trace=False, **inputs):
    """Full pipeline; returns (output, BassKernelResults)."""
    in_maps, per_core_idx, shape, t_cap, ln_affine = _prepare(**inputs)
    nc = _get_program(t_cap, ln_affine)
    kw = {}
    if trace:
        kw = dict(trace=True, trace_cores=list(range(N_CORES)))
    res = run_bass_kernel_spmd(nc, in_maps, core_ids=list(range(N_CORES)), **kw)
    return _scatter(res.results, per_core_idx, shape), res


def kernel(**inputs):
    try:
        out, _ = run(trace=False, **inputs)
    except Exception:
        # transient device errors (e.g. NRT_EXEC_UNIT_UNRECOVERABLE) clear
        # on a fresh attempt
        out, _ = run(trace=False, **inputs)
    return out

